# revision 16
# baseline (speedup 1.0000x reference)
"""BatchAllTripletLoss on 8 Trainium2 NeuronCores.

Strategy (data-parallel over anchors x negative-columns):
  - Host: sort the batch by label (loss is permutation invariant). After the
    sort every anchor's positives live in a contiguous run of columns within
    +/-(W-1) of its own column (W = max class size).
  - 8 cores = 4 anchor blocks (128 anchors) x 2 negative halves (256 cols).
    Each core computes distances for its block via PE matmul (fp16 inputs,
    fp32 accumulate), masks positives/negatives from labels, extracts the
    per-anchor positive band with a diagonal-stride DMA, and then runs the
    hot loop: for each band position j, one fused instruction computes
    sum_n relu(pos_j - d_neg) (and one counts pos_j - d_neg > eps),
    accumulated per-anchor via the instruction's accum_out. Work is split
    across the Vector and Scalar engines.
  - Host combines 8x[128,4] partials into the two output scalars.
"""
import sys
sys.path.insert(0, "/opt/trn_rl_repo")

import numpy as np
from contextlib import ExitStack

import bass_rust
import concourse.bass as bass
import concourse.tile as tile
from concourse import bacc, mybir
from concourse.bass_utils import run_bass_kernel_spmd

F32 = mybir.dt.float32
F16 = mybir.dt.float16
Alu = mybir.AluOpType
Act = mybir.ActivationFunctionType
AX = mybir.AxisListType

B = 512          # batch
P = 128          # anchors per block (partition dim)
NH = 256         # negative columns per core (half of B)
LARGE = 1.0e6
EPS_TL = 1.0e-5  # reference's tl > 1e-5 threshold
N_CORES = 8

_cache = {}


def _build(W: int, loop_iters: int | None = None):
    """Build + compile the per-core Bass program for max class size W.

    loop_iters: if set, wrap the whole body in a hardware For_i loop
    (benchmarking only - out is rewritten every iteration)."""
    WB = 2 * W - 1            # positive band width per anchor
    WWIN = P + 2 * (W - 1)    # window columns covering all positives of a block
    # split the band between DVE (scalar_tensor_tensor) and ACT (activation)
    n_act = max(1, int(round(WB * 327.0 / (327.0 + 585.0))))
    n_dve = WB - n_act

    nc = bacc.Bacc("TRN2", target_bir_lowering=False, debug=False,
                   num_devices=N_CORES)

    lhsT_d = nc.dram_tensor("lhsT", [P, (B // P) * P], F16, kind="ExternalInput")
    rhsn_d = nc.dram_tensor("rhsn", [P, (B // P) * NH], F16, kind="ExternalInput")
    rhsw_d = nc.dram_tensor("rhsw", [P, (B // P) * WWIN], F16, kind="ExternalInput")
    laba_d = nc.dram_tensor("laba", [P, 1], F32, kind="ExternalInput")
    labn_d = nc.dram_tensor("labn", [1, NH], F32, kind="ExternalInput")
    labw_d = nc.dram_tensor("labw", [1, WWIN], F32, kind="ExternalInput")
    idlp_d = nc.dram_tensor("idlp", [P, WWIN], F32, kind="ExternalInput")
    out_d = nc.dram_tensor("out", [P, 4], F32, kind="ExternalOutput")

    with tile.TileContext(nc) as tc, ExitStack() as ctx:
        pool = ctx.enter_context(tc.tile_pool(name="sbuf", bufs=2))
        spool = ctx.enter_context(tc.tile_pool(name="scr", bufs=3))
        ppool = ctx.enter_context(tc.tile_pool(name="psum", bufs=1, space="PSUM"))
        dpool = ctx.enter_context(tc.tile_pool(name="dram", bufs=1, space="DRAM"))

        K = B // P  # contraction chunks

        def _body():
            # ---- loads ----
            # host pre-interleaves rows (chunk-major per partition) so each
            # tensor arrives in ONE DMA of 128 long contiguous lines; chunk k
            # of the contraction is then tile[:, k, :].
            lhsT_t = pool.tile([P, K, P], F16)
            rhsn_t = pool.tile([P, K, NH], F16)
            rhsw_t = pool.tile([P, K, WWIN], F16)
            # w-side chain is the longest: load rhsw first
            nc.sync.dma_start(rhsw_t[:], rhsw_d.ap())
            nc.sync.dma_start(lhsT_t[:], lhsT_d.ap())
            nc.sync.dma_start(rhsn_t[:], rhsn_d.ap())
            lhsT = [lhsT_t[:, k, :] for k in range(K)]
            rhsn = [rhsn_t[:, k, :] for k in range(K)]
            rhsw = [rhsw_t[:, k, :] for k in range(K)]
            laba = pool.tile([P, 1], F32)
            nc.sync.dma_start(laba[:], laba_d.ap())
            labn = pool.tile([1, NH], F32)
            nc.sync.dma_start(labn[:], labn_d.ap())
            labw = pool.tile([1, WWIN], F32)
            nc.sync.dma_start(labw[:], labw_d.ap())
            idlp = pool.tile([P, WWIN], F32)
            nc.sync.dma_start(idlp[:], idlp_d.ap())

            ones_r = pool.tile([1, P], F32)
            nc.vector.memset(ones_r[:], 1.0)
            ones_c = pool.tile([P, 1], F32)
            nc.vector.memset(ones_c[:], 1.0)
            zero_n = pool.tile([P, NH], F32)
            nc.vector.memset(zero_n[:], 0.0)
            zero_w = pool.tile([P, WWIN], F32)
            nc.vector.memset(zero_w[:], 0.0)

            # ---- column norms (squares + ones-matmul) ----
            def col_norms(rhs_chunks, width, tag):
                ps = ppool.tile([1, width], F32, tag=f"ps{tag}", name=f"sqps{tag}")
                for k in range(K):
                    sq = spool.tile([P, width], F32, tag=f"sq{tag}",
                                    name=f"sq{tag}_{k}")
                    nc.vector.tensor_tensor(
                        out=sq[:], in0=rhs_chunks[k], in1=rhs_chunks[k],
                        op=Alu.mult)
                    nc.tensor.matmul(ps[:], ones_c[:], sq[:],
                                     start=(k == 0), stop=(k == K - 1))
                row = pool.tile([1, width], F32, tag=f"sqrow{tag}",
                                name=f"sqrow{tag}")
                nc.vector.tensor_copy(row[:], ps[:])
                return row

            sqw_row = col_norms(rhsw, WWIN, "w")

            # ---- anchor norms: transpose sqw_row[self cols] via tiny DMA RT ----
            sqd = dpool.tile([1, WWIN], F32)
            nc.sync.dma_start(sqd[:], sqw_row[:])
            sq_src = sqd[:].copy()
            sq_src.ap = bass_rust.VecI64Pair([[1, P], [1, 1]])
            sq_src.offset = sq_src.offset + (W - 1)
            sq_a = pool.tile([P, 1], F32)
            nc.sync.dma_start(sq_a[:], sq_src)

            # ---- scaled lhsT (-2x) ----
            lhsTm2 = [pool.tile([P, P], F16, tag=f"lm2{k}", name=f"lm2{k}")
                      for k in range(K)]
            for k in range(K):
                nc.vector.tensor_scalar_mul(lhsTm2[k][:], lhsT[k], -2.0)

            # ---- gram + col-norm row accumulated on PE,
            # then d2 = max(psum + sq_a, 0), d = sqrt ----
            def dist(rhs_chunks, sq_row, width, tag):
                g = ppool.tile([P, width], F32, tag=f"g{tag}", name=f"g{tag}")
                for k in range(K):
                    nc.tensor.matmul(g[:], lhsTm2[k][:], rhs_chunks[k],
                                     start=(k == 0), stop=False)
                nc.tensor.matmul(g[:], ones_r[:], sq_row[:],
                                 start=False, stop=True)
                d2c = spool.tile([P, width], F32, tag=f"d2c{tag}",
                                 name=f"d2c{tag}")
                nc.vector.tensor_scalar(
                    out=d2c[:], in0=g[:], scalar1=sq_a[:], scalar2=0.0,
                    op0=Alu.add, op1=Alu.max)
                d = pool.tile([P, width], F32, tag=f"d{tag}", name=f"d{tag}")
                nc.scalar.activation(d[:], d2c[:], Act.Sqrt)
                return d

            d_w = dist(rhsw, sqw_row, WWIN, "w")

            # ---- label masks ----
            def lab_bcast(lab_row, width, tag):
                ps = ppool.tile([P, width], F32, tag=f"ps{tag}", name=f"lb{tag}")
                nc.tensor.matmul(ps[:], ones_r[:], lab_row[:],
                                 start=True, stop=True)
                return ps

            labn_b = lab_bcast(labn, NH, "n")
            eq_n = pool.tile([P, NH], F32)
            nc.vector.scalar_tensor_tensor(
                out=eq_n[:], in0=labn_b[:], scalar=laba[:], in1=zero_n[:],
                op0=Alu.is_equal, op1=Alu.add)

            labw_b = lab_bcast(labw, WWIN, "w")
            eq_w = pool.tile([P, WWIN], F32)
            csize = pool.tile([P, 1], F32)
            nc.vector.scalar_tensor_tensor(
                out=eq_w[:], in0=labw_b[:], scalar=laba[:], in1=zero_w[:],
                op0=Alu.is_equal, op1=Alu.add, accum_out=csize[:])

            # ---- positives window: DPw = d + (eq-1)*LARGE - 2*LARGE*self ----
            t_w = spool.tile([P, WWIN], F32, tag="tw")
            nc.vector.scalar_tensor_tensor(
                out=t_w[:], in0=eq_w[:], scalar=LARGE, in1=d_w[:],
                op0=Alu.mult, op1=Alu.add)
            dpw = pool.tile([P, WWIN], F32)
            nc.vector.tensor_tensor(out=dpw[:], in0=t_w[:], in1=idlp[:],
                                    op=Alu.subtract)

            # ---- band extraction via diagonal-stride DMA ----
            dpd = dpool.tile([P, WWIN], F32)
            nc.sync.dma_start(dpd[:], dpw[:])
            band_src = dpd[:].copy()
            band_src.ap = bass_rust.VecI64Pair([[WWIN + 1, P], [1, WB]])
            pos = pool.tile([P, WB], F32)
            nc.sync.dma_start(pos[:], band_src)
            pos_e = pool.tile([P, WB], F32)
            nc.vector.tensor_scalar_sub(pos_e[:], pos[:], EPS_TL)

            # ---- n-side (shorter chain): fills the band-RT wait gap ----
            sqn_row = col_norms(rhsn, NH, "n")
            d_n = dist(rhsn, sqn_row, NH, "n")
            ndn = pool.tile([P, NH], F32)
            nc.vector.scalar_tensor_tensor(
                out=ndn[:], in0=eq_n[:], scalar=-LARGE, in1=d_n[:],
                op0=Alu.mult, op1=Alu.subtract)

            # ---- hot loop ----
            sum_d = pool.tile([P, max(n_dve, 1)], F32)
            cnt_d = pool.tile([P, max(n_dve, 1)], F32)
            sum_a = pool.tile([P, max(n_act, 1)], F32)
            sgn_a = pool.tile([P, max(n_act, 1)], F32)
            if n_dve == 0:
                nc.vector.memset(sum_d[:], 0.0)
                nc.vector.memset(cnt_d[:], 0.0)

            jd = ja = 0
            for j in range(WB):
                use_act = (j * n_act) // WB != ((j + 1) * n_act) // WB
                if use_act:
                    scr1 = ppool.tile([P, NH], F32, tag="ascr",
                                      name=f"ascr1_{j}", bufs=2)
                    nc.scalar.activation(scr1[:], ndn[:], Act.Relu,
                                         bias=pos[:, j:j + 1], scale=1.0,
                                         accum_out=sum_a[:, ja:ja + 1])
                    scr2 = ppool.tile([P, NH], F32, tag="ascr",
                                      name=f"ascr2_{j}", bufs=2)
                    nc.scalar.activation(scr2[:], ndn[:], Act.Sign,
                                         bias=pos_e[:, j:j + 1], scale=1.0,
                                         accum_out=sgn_a[:, ja:ja + 1])
                    ja += 1
                else:
                    scr1 = spool.tile([P, NH], F32, tag="dscr",
                                      name=f"dscr1_{j}")
                    nc.vector.scalar_tensor_tensor(
                        out=scr1[:], in0=ndn[:], scalar=pos[:, j:j + 1],
                        in1=zero_n[:], op0=Alu.add, op1=Alu.max,
                        accum_out=sum_d[:, jd:jd + 1])
                    scr2 = spool.tile([P, NH], F32, tag="dscr",
                                      name=f"dscr2_{j}")
                    nc.vector.scalar_tensor_tensor(
                        out=scr2[:], in0=ndn[:], scalar=pos_e[:, j:j + 1],
                        in1=zero_n[:], op0=Alu.add, op1=Alu.is_gt,
                        accum_out=cnt_d[:, jd:jd + 1])
                    jd += 1
            assert ja == n_act and jd == n_dve

            # ---- final reductions ----
            out_t = pool.tile([P, 4], F32)
            r_sum_d = pool.tile([P, 1], F32)
            nc.vector.tensor_reduce(out=r_sum_d[:], in_=sum_d[:], axis=AX.X,
                                    op=Alu.add)
            r_sum_a = pool.tile([P, 1], F32)
            nc.vector.tensor_reduce(out=r_sum_a[:], in_=sum_a[:], axis=AX.X,
                                    op=Alu.add)
            nc.vector.tensor_tensor(out=out_t[:, 0:1], in0=r_sum_d[:],
                                    in1=r_sum_a[:], op=Alu.add)

            r_cnt_d = pool.tile([P, 1], F32)
            nc.vector.tensor_reduce(out=r_cnt_d[:], in_=cnt_d[:], axis=AX.X,
                                    op=Alu.add)
            r_sgn = pool.tile([P, 1], F32)
            nc.vector.tensor_reduce(out=r_sgn[:], in_=sgn_a[:], axis=AX.X,
                                    op=Alu.add)
            r_cnt_a = pool.tile([P, 1], F32)
            nc.vector.tensor_scalar(
                out=r_cnt_a[:], in0=r_sgn[:], scalar1=0.5,
                scalar2=float(NH // 2 * n_act), op0=Alu.mult, op1=Alu.add)
            nc.vector.tensor_tensor(out=out_t[:, 1:2], in0=r_cnt_d[:],
                                    in1=r_cnt_a[:], op=Alu.add)

            pc = pool.tile([P, 1], F32)
            nc.vector.tensor_scalar_sub(pc[:], csize[:], 1.0)
            nn_ = pool.tile([P, 1], F32)
            nc.vector.tensor_scalar(
                out=nn_[:], in0=csize[:], scalar1=-1.0, scalar2=float(B),
                op0=Alu.mult, op1=Alu.add)
            nc.vector.tensor_tensor(out=out_t[:, 2:3], in0=pc[:], in1=nn_[:],
                                    op=Alu.mult)
            nc.vector.tensor_copy(out_t[:, 3:4], csize[:])

            nc.sync.dma_start(out_d.ap(), out_t[:])

        if loop_iters is None:
            _body()
        else:
            with tc.For_i(0, loop_iters, 1):
                _body()

    nc.compile()
    return nc


def _ilv(a):
    """[512, x] -> [128, 4*x]: partition p holds rows p, p+128, p+256, p+384."""
    x = a.shape[1]
    return np.ascontiguousarray(
        a.reshape(4, P, x).transpose(1, 0, 2).reshape(P, 4 * x))


def _prepare(embeddings: np.ndarray, labels: np.ndarray):
    emb = np.ascontiguousarray(np.asarray(embeddings, dtype=np.float32))
    lab = np.asarray(labels)

    perm = np.argsort(lab, kind="stable")
    e_p = emb[perm]
    lab_p = lab[perm].astype(np.float32)

    _, counts = np.unique(lab_p, return_counts=True)
    W = int(counts.max())
    WWIN = P + 2 * (W - 1)

    e_pT = np.ascontiguousarray(e_p.T.astype(np.float16))   # [512 (d), 512 (x)]
    pad = W - 1
    e_padT = np.zeros((B, B + 2 * pad), dtype=np.float16)
    e_padT[:, pad:pad + B] = e_pT
    lab_pad = np.full((B + 2 * pad,), -1.0, dtype=np.float32)
    lab_pad[pad:pad + B] = lab_p

    # combined mask constant: LARGE everywhere + extra 2*LARGE on the
    # window-local self column (w == a + W - 1); same for every core.
    idlp = np.full((P, WWIN), LARGE, dtype=np.float32)
    for a in range(P):
        idlp[a, a + W - 1] += 2.0 * LARGE

    in_maps = []
    for c in range(N_CORES):
        b, h = c >> 1, c & 1
        bs = b * P
        in_maps.append({
            "lhsT": _ilv(e_pT[:, bs:bs + P]),
            "rhsn": _ilv(e_pT[:, h * NH:(h + 1) * NH]),
            "rhsw": _ilv(e_padT[:, bs:bs + WWIN]),
            "laba": np.ascontiguousarray(lab_p[bs:bs + P].reshape(P, 1)),
            "labn": np.ascontiguousarray(
                lab_p[h * NH:(h + 1) * NH].reshape(1, NH)),
            "labw": np.ascontiguousarray(lab_pad[bs:bs + WWIN].reshape(1, WWIN)),
            "idlp": idlp,
        })
    return W, in_maps


def _combine(outs):
    """outs: list of 8 [128, 4] arrays -> (loss, fraction_positive)."""
    loss_sum = 0.0
    num_pos = 0.0
    num_valid = 0.0
    for c in range(N_CORES):
        o = np.asarray(outs[c], dtype=np.float64)
        loss_sum += o[:, 0].sum()
        num_pos += o[:, 1].sum()
        if (c & 1) == 0:
            num_valid += o[:, 2].sum()
    loss = np.float32(loss_sum / (num_pos + 1e-5))
    frac = np.float32(num_pos / (num_valid + 1e-5))
    return (loss, frac)


def kernel(embeddings: np.ndarray, labels: np.ndarray):
    W, in_maps = _prepare(embeddings, labels)
    if W not in _cache:
        _cache[W] = _build(W)
    nc = _cache[W]
    res = run_bass_kernel_spmd(nc, in_maps, core_ids=list(range(N_CORES)))
    return _combine([res.results[c]["out"] for c in range(N_CORES)])
